# revision 12
# baseline (speedup 1.0000x reference)
"""Trainium2 Bass kernel for nn_SamplePolicy_14886356648064.

Reference semantics (T=4 resample rounds, K=4 vote threshold, H=8 heads):
  each round: per-head argmax over src -> presence vector per head ->
  counting = sum of presence over heads -> trigger = counting.max() <= K ->
  if trigger, replace all heads with head `sampled_t` (broadcast).

Exact algebraic collapse of the T-loop: only round 0's trigger and sampled_0
matter, for ANY input -> output is either the input (no trigger) or
broadcast(aw[3]).  sampled_0 = jax.random.randint(fold_in(key(42), 0), (),
0, 7) == 3 (threefry, platform independent).

Device work: per-row 128-wide block maxima of the fp16-cast input, one head
per core. fp16 rounding is monotone (x <= y => f16(x) <= f16(y)), so the f32
row argmax always lives in a block whose fp16 block-max ties the row's fp16
max. The device streams the 16MB fp16 head slice once; the host then scans
only the tied candidate blocks (~2 of 32 per row for uniform data) in f32
for the exact first-occurrence argmax, and runs the tiny vote logic.

Measured HW model driving the design (all 128 partitions in parallel):
  DMA: two HWDGE rings, ~425 GB/s aggregate when both active (sync ring
    ~236, scalar ring ~189); 16 DMA engines at ~26.8 GB/s each, 100% busy.
  DVE tensor_reduce: 1.04 ns/input-elem + 150ns, ANY dtype (no fast mode).
  DVE tensor_tensor max: 0.52 ns/output-elem + 150ns for packed 2-byte
    dtypes (2x_1p mode); pool/scalar engines cannot run tensor ops on TRN2.
Block maxima = fold tree (pairwise tensor_max halving block width 4x) + one
small reduce: 5.3us per 2MB chunk vs 9.1us for direct reduces.

Layout: head slice [2048, 4096] fp16 viewed as x[p, k] = [128, 65536]
(partition p holds rows 16p..16p+15 contiguously; k = t*4096 + c). Chunks
are arbitrary [c0, c0+clen) column ranges - no tile boundaries - and the
block-max output bm[p, t*32+b] reshapes straight to row-major [2048, 32].
The full 16MB fits in SBUF (128KB/partition), so every DMA job owns a
dedicated region: no write-after-read hazards anywhere.

Schedule: ring sizes/order chosen by an offline search against the
calibrated two-ring rate model so chunk completions (ring-FIFO) match the
vector engine's consumption order with minimal starvation under +-10% ring
rate skew. Small chunks first (vector starts ~10.5us), one big final chunk
(vector is backlogged at the end anyway; big chunks cost least per element).
"""

import numpy as np

H = 8
TGT = 2048
SRC = 4096
P = 128            # SBUF partitions
RPP = 16           # rows per partition
W = RPP * SRC      # 65536 elements per partition
NBLK = 32          # block maxima per original row
BLK = SRC // NBLK  # 128
NBLK_ALL = W // BLK  # 512 block maxima per partition
K_THRESH = 4
SAMPLED_T0 = 3

# (ring, clen) in VECTOR (predicted-arrival) order; c0 = running sum.
# The two HWDGE queues split DMA-engine slots round-robin per DESCRIPTOR, so
# bytes split proportional to descriptor size; a lone queue streams ~330-400
# GB/s, both together ~425. Ring A opens with four 1MB chunks streamed solo
# (ring B's first issue is semaphore-gated on A1's completion) so the vector
# engine starts folding ~12.5us; the rings then interleave 2MB chunks with a
# half-chunk offset (no lumpy pair completions).
SCHEDULE = [
    ("A", 4096), ("A", 4096), ("A", 4096), ("A", 4096),
    ("B", 8192), ("A", 8192), ("B", 8192), ("A", 8192),
    ("B", 8192), ("B", 8192),
]
GATE_B_ON = 0      # ring B's first dma_start waits for this job's s_in
# fold depth per chunk size (then one reduce over the remaining width)
FOLDK = {16384: 4, 8192: 4, 4096: 3, 2048: 2, 1024: 1}
# gpsimd tick train: engines re-evaluate blocked semaphore waits only when
# some new semaphore update arrives; a steady stream of dummy sem_incs
# bounds that wake-up latency at one tick period (~0.5us).
TICKS = 95
TICK_SETS = 1

JOBS = []          # (ring, c0, clen) in vector order
_g = 0
for _ring, _clen in SCHEDULE:
    JOBS.append((_ring, _g, _clen))
    _g += _clen
assert _g == W
NJOBS = len(JOBS)
FINAL_C0 = JOBS[-1][1]          # bulk store covers blocks [0, FINAL_C0/BLK)

_cache = {}


def _build_nc():
    """Raw Bass program, one head per core."""
    from contextlib import ExitStack

    import concourse.bass as bass
    import concourse.mybir as mybir

    nc = bass.Bass()
    f16 = mybir.dt.float16
    x = nc.declare_dram_parameter("x", [P, W], f16, isOutput=False)
    # bm[p, t*NBLK + b] = max of block b of row 16p + t
    bm = nc.declare_dram_parameter("bm", [P, NBLK_ALL], f16, isOutput=True)

    with ExitStack() as ctx:
        tiles = ctx.enter_context(nc.sbuf_tensor([P, W], f16))
        scra = ctx.enter_context(nc.sbuf_tensor([P, 8192], f16))
        scrb = ctx.enter_context(nc.sbuf_tensor([P, 4096], f16))
        bmsb = ctx.enter_context(nc.sbuf_tensor([P, NBLK_ALL], f16))
        tick = ctx.enter_context(nc.sbuf_tensor([1, 512], f16))
        s_in = [ctx.enter_context(nc.semaphore(f"s_in{j}")) for j in range(NJOBS)]
        s_red = ctx.enter_context(nc.semaphore("s_red"))
        s_out = ctx.enter_context(nc.semaphore("s_out"))
        s_tick = ctx.enter_context(nc.semaphore("s_tick"))
        block = ctx.enter_context(nc.Block())

        def issue_loads(eng, ring):
            first = True
            for j, (r, c0, clen) in enumerate(JOBS):
                if r != ring:
                    continue
                if ring == "B" and first:
                    eng.wait_ge(s_in[GATE_B_ON], 16)
                    first = False
                eng.dma_start(
                    out=tiles[:, c0 : c0 + clen],
                    in_=x[:, c0 : c0 + clen],
                ).then_inc(s_in[j], 16)

        @block.sync
        def _(sync):
            issue_loads(sync, "A")
            sync.wait_ge(s_out, 32)

        @block.scalar
        def _(scalar):
            issue_loads(scalar, "B")
            # bulk store overlaps the final chunk's fold train
            scalar.wait_ge(s_red, NJOBS - 1)
            scalar.dma_start(
                out=bm[:, : FINAL_C0 // BLK], in_=bmsb[:, : FINAL_C0 // BLK]
            ).then_inc(s_out, 16)
            scalar.wait_ge(s_red, NJOBS)
            scalar.dma_start(
                out=bm[:, FINAL_C0 // BLK :], in_=bmsb[:, FINAL_C0 // BLK :]
            ).then_inc(s_out, 16)

        @block.gpsimd
        def _(gpsimd):
            # pure ticker: never waits, just emits a sem update every ~0.5us
            for _ in range(TICKS):
                for _ in range(TICK_SETS):
                    nc.gpsimd.memset(tick[:, :], 0.0)
                gpsimd.sem_inc(s_tick, 1)

        @block.vector
        def _(vector):
            for j, (r, c0, clen) in enumerate(JOBS):
                vector.wait_ge(s_in[j], 16)
                nblk = clen // BLK
                # fold tree: halve block width FOLDK times, ping-pong scratch
                src = tiles[:, c0 : c0 + clen]
                w = BLK
                for step in range(FOLDK[clen]):
                    dst = (scra if step % 2 == 0 else scrb)[:, : nblk * w // 2]
                    sv = src.rearrange("p (b c) -> p b c", c=w)
                    nc.vector.tensor_max(
                        out=dst.rearrange("p (b c) -> p b c", c=w // 2),
                        in0=sv[:, :, : w // 2],
                        in1=sv[:, :, w // 2 :],
                    )
                    src, w = dst, w // 2
                # finish with one reduce over the remaining width
                nc.vector.reduce_max(
                    out=bmsb[:, c0 // BLK : (c0 + clen) // BLK],
                    in_=src.rearrange("p (b c) -> p b c", c=w),
                    axis=mybir.AxisListType.X,
                ).then_inc(s_red, 1)

    return nc


def _get_nc():
    if "nc" not in _cache:
        _cache["nc"] = _build_nc()
    return _cache["nc"]


def run_device(aw, **run_kwargs):
    """Run the per-head fp16 block-max kernel on 8 cores.

    Takes the full f32 [H, TGT, SRC] tensor; returns ([H, TGT, NBLK] fp16
    block maxima of the fp16-cast data, results).
    """
    from concourse.bass_utils import run_bass_kernel_spmd

    nc = _get_nc()
    aw16 = aw.astype(np.float16)
    in_maps = [
        {"x": np.ascontiguousarray(aw16[c]).reshape(P, W)} for c in range(H)
    ]
    res = run_bass_kernel_spmd(nc, in_maps, list(range(H)), **run_kwargs)
    # bm[p, t*NBLK+b] -> row-major [TGT, NBLK]: row = 16p + t
    bms = [res.results[c]["bm"].reshape(TGT, NBLK) for c in range(H)]
    return np.stack(bms), res


def _exact_argmax(aw, bm):
    """Exact first-occurrence np.argmax(aw, -1) from fp16 block maxima.

    fp16 rounding is monotone, so every element equal to the f32 row max
    lives in a block whose fp16 block max ties the row's fp16 max. Scanning
    the tied blocks in ascending order preserves first-occurrence order.
    """
    rowmax = bm.max(-1, keepdims=True)
    mask = bm == rowmax  # [H, TGT, NBLK] candidate blocks
    cmax = int(mask.sum(-1).max())
    # candidate block indices in ascending order, padded with non-candidates
    order = np.argsort(~mask, axis=-1, kind="stable")[..., :cmax]
    valid = np.take_along_axis(mask, order, -1)
    blocks = aw.reshape(H, TGT, NBLK, BLK)
    win = np.take_along_axis(blocks, order[..., None], axis=2)  # [H,T,cmax,BLK]
    win = np.where(valid[..., None], win, -np.inf).reshape(H, TGT, cmax * BLK)
    j = win.argmax(-1)
    b = np.take_along_axis(order, (j // BLK)[..., None], -1)[..., 0]
    return b * BLK + j % BLK


def kernel(attention_weight):
    aw = np.asarray(attention_weight)
    assert aw.shape == (H, TGT, SRC), aw.shape
    aw = aw.astype(np.float32, copy=False)

    try:
        bm, _ = run_device(aw)
    except Exception as e:  # device path failed: fall back to host blockmax
        import traceback

        traceback.print_exc()
        print(f"WARNING: device path failed ({e!r}); falling back to numpy")
        bm = aw.astype(np.float16).reshape(H, TGT, NBLK, BLK).max(-1)

    cand = _exact_argmax(aw, bm)  # [H, TGT]
    present = np.zeros((H, SRC), np.float32)
    present[np.arange(H)[:, None], cand] = 1.0
    counting = present.sum(axis=0)

    if counting.max() <= K_THRESH:
        return np.broadcast_to(aw[SAMPLED_T0], aw.shape).copy()
    return aw


# revision 15
# speedup vs baseline: 1.0259x; 1.0259x over previous
"""Trainium2 Bass kernel for nn_SamplePolicy_14886356648064.

Reference semantics (T=4 resample rounds, K=4 vote threshold, H=8 heads):
  each round: per-head argmax over src -> presence vector per head ->
  counting = sum of presence over heads -> trigger = counting.max() <= K ->
  if trigger, replace all heads with head `sampled_t` (broadcast).

Exact algebraic collapse of the T-loop: only round 0's trigger and sampled_0
matter, for ANY input -> output is either the input (no trigger) or
broadcast(aw[3]).  sampled_0 = jax.random.randint(fold_in(key(42), 0), (),
0, 7) == 3 (threefry, platform independent).

Device work: per-row 128-wide block maxima of the fp16-cast input, one head
per core. fp16 rounding is monotone (x <= y => f16(x) <= f16(y)), so the f32
row argmax always lives in a block whose fp16 block-max ties the row's fp16
max. The device streams the 16MB fp16 head slice once; the host then scans
only the tied candidate blocks (~2 of 32 per row for uniform data) in f32
for the exact first-occurrence argmax, and runs the tiny vote logic.

Measured HW model driving the design (all 128 partitions in parallel):
  DMA: two HWDGE rings, ~425 GB/s aggregate when both active (sync ring
    ~236, scalar ring ~189); 16 DMA engines at ~26.8 GB/s each, 100% busy.
  DVE tensor_reduce: 1.04 ns/input-elem + 150ns, ANY dtype (no fast mode).
  DVE tensor_tensor max: 0.52 ns/output-elem + 150ns for packed 2-byte
    dtypes (2x_1p mode); pool/scalar engines cannot run tensor ops on TRN2.
Block maxima = fold tree (pairwise tensor_max halving block width 4x) + one
small reduce: 5.3us per 2MB chunk vs 9.1us for direct reduces.

Layout: head slice [2048, 4096] fp16 viewed as x[p, k] = [128, 65536]
(partition p holds rows 16p..16p+15 contiguously; k = t*4096 + c). Chunks
are arbitrary [c0, c0+clen) column ranges - no tile boundaries - and the
block-max output bm[p, t*32+b] reshapes straight to row-major [2048, 32].
The full 16MB fits in SBUF (128KB/partition), so every DMA job owns a
dedicated region: no write-after-read hazards anywhere.

Schedule: ring sizes/order chosen by an offline search against the
calibrated two-ring rate model so chunk completions (ring-FIFO) match the
vector engine's consumption order with minimal starvation under +-10% ring
rate skew. Small chunks first (vector starts ~10.5us), one big final chunk
(vector is backlogged at the end anyway; big chunks cost least per element).
"""

import numpy as np

H = 8
TGT = 2048
SRC = 4096
P = 128            # SBUF partitions
RPP = 16           # rows per partition
W = RPP * SRC      # 65536 elements per partition
NBLK = 32          # block maxima per original row
BLK = SRC // NBLK  # 128
NBLK_ALL = W // BLK  # 512 block maxima per partition
K_THRESH = 4
SAMPLED_T0 = 3

# (ring, clen) in VECTOR (predicted-arrival) order; c0 = running sum.
# The two HWDGE queues split DMA-engine slots round-robin per DESCRIPTOR, so
# bytes split proportional to descriptor size; a lone queue streams ~330-400
# GB/s, both together ~425. Ring A opens with four 1MB chunks streamed solo
# (ring B's first issue is semaphore-gated on A1's completion) so the vector
# engine starts folding ~12.5us; the rings then interleave 2MB chunks with a
# half-chunk offset (no lumpy pair completions).
SCHEDULE = [
    ("A", 4096), ("A", 4096), ("A", 4096), ("A", 4096),
    ("B", 8192), ("A", 8192), ("B", 8192), ("A", 8192),
    ("B", 8192), ("B", 8192),
]
GATE_B_ON = 1      # ring B's first dma_start waits for this job's load sem
# fold depth per chunk size (then one reduce over the remaining width)
FOLDK = {16384: 4, 8192: 4, 4096: 3, 2048: 2, 1024: 1}
# gpsimd tick train: engines re-evaluate blocked semaphore waits only when
# some new semaphore update arrives; a steady stream of dummy sem_incs
# bounds that wake-up latency at one tick period (~0.5us).
TICKS = 95
TICK_SETS = 1

JOBS = []          # (ring, c0, clen) in vector order
_g = 0
for _ring, _clen in SCHEDULE:
    JOBS.append((_ring, _g, _clen))
    _g += _clen
assert _g == W
NJOBS = len(JOBS)
FINAL_C0 = JOBS[-1][1]          # bulk store covers blocks [0, FINAL_C0/BLK)

_cache = {}


def _build_nc():
    """Raw Bass program, one head per core."""
    from contextlib import ExitStack

    import concourse.bass as bass
    import concourse.mybir as mybir

    nc = bass.Bass()
    f16 = mybir.dt.float16
    x = nc.declare_dram_parameter("x", [P, W], f16, isOutput=False)
    # bm[p, t*NBLK + b] = max of block b of row 16p + t
    bm = nc.declare_dram_parameter("bm", [P, NBLK_ALL], f16, isOutput=True)

    with ExitStack() as ctx:
        tiles = ctx.enter_context(nc.sbuf_tensor([P, W], f16))
        scra = ctx.enter_context(nc.sbuf_tensor([P, 8192], f16))
        scrb = ctx.enter_context(nc.sbuf_tensor([P, 4096], f16))
        bmsb = ctx.enter_context(nc.sbuf_tensor([P, NBLK_ALL], f16))
        tick = ctx.enter_context(nc.sbuf_tensor([1, 512], f16))
        # one load semaphore per RING, cumulative thresholds (16 per job):
        # a threshold of 16*k implies the ring's first k jobs all completed,
        # regardless of increment interleaving across jobs.
        s_ld = {r: ctx.enter_context(nc.semaphore(f"s_ld{r}")) for r in "AB"}
        s_red = ctx.enter_context(nc.semaphore("s_red"))
        s_out = ctx.enter_context(nc.semaphore("s_out"))
        s_tick = ctx.enter_context(nc.semaphore("s_tick"))
        block = ctx.enter_context(nc.Block())

        # per-job (ring, #jobs on that ring up to and including this one)
        RANK = []
        cnt = {"A": 0, "B": 0}
        for r, _, _ in JOBS:
            cnt[r] += 1
            RANK.append((r, cnt[r]))
        gate_ring, gate_rank = RANK[GATE_B_ON]

        def issue_loads(eng, ring):
            first = True
            for j, (r, c0, clen) in enumerate(JOBS):
                if r != ring:
                    continue
                if ring == "B" and first:
                    eng.wait_ge(s_ld[gate_ring], 16 * gate_rank)
                    first = False
                eng.dma_start(
                    out=tiles[:, c0 : c0 + clen],
                    in_=x[:, c0 : c0 + clen],
                ).then_inc(s_ld[r], 16)

        @block.sync
        def _(sync):
            issue_loads(sync, "A")
            sync.wait_ge(s_out, 32)

        @block.scalar
        def _(scalar):
            issue_loads(scalar, "B")
            # bulk store overlaps the final chunk's fold train
            scalar.wait_ge(s_red, NJOBS - 1)
            scalar.dma_start(
                out=bm[:, : FINAL_C0 // BLK], in_=bmsb[:, : FINAL_C0 // BLK]
            ).then_inc(s_out, 16)
            scalar.wait_ge(s_red, NJOBS)
            scalar.dma_start(
                out=bm[:, FINAL_C0 // BLK :], in_=bmsb[:, FINAL_C0 // BLK :]
            ).then_inc(s_out, 16)

        @block.gpsimd
        def _(gpsimd):
            # pure ticker: never waits, just emits a sem update every ~0.5us
            for _ in range(TICKS):
                for _ in range(TICK_SETS):
                    nc.gpsimd.memset(tick[:, :], 0.0)
                gpsimd.sem_inc(s_tick, 1)

        @block.vector
        def _(vector):
            for j, (r, c0, clen) in enumerate(JOBS):
                vector.wait_ge(s_ld[r], 16 * RANK[j][1])
                nblk = clen // BLK
                # fold tree: halve block width FOLDK times, ping-pong scratch
                src = tiles[:, c0 : c0 + clen]
                w = BLK
                for step in range(FOLDK[clen]):
                    dst = (scra if step % 2 == 0 else scrb)[:, : nblk * w // 2]
                    sv = src.rearrange("p (b c) -> p b c", c=w)
                    nc.vector.tensor_max(
                        out=dst.rearrange("p (b c) -> p b c", c=w // 2),
                        in0=sv[:, :, : w // 2],
                        in1=sv[:, :, w // 2 :],
                    )
                    src, w = dst, w // 2
                # finish with one reduce over the remaining width
                nc.vector.reduce_max(
                    out=bmsb[:, c0 // BLK : (c0 + clen) // BLK],
                    in_=src.rearrange("p (b c) -> p b c", c=w),
                    axis=mybir.AxisListType.X,
                ).then_inc(s_red, 1)

    return nc


def _get_nc():
    if "nc" not in _cache:
        _cache["nc"] = _build_nc()
    return _cache["nc"]


def run_device(aw, **run_kwargs):
    """Run the per-head fp16 block-max kernel on 8 cores.

    Takes the full f32 [H, TGT, SRC] tensor; returns ([H, TGT, NBLK] fp16
    block maxima of the fp16-cast data, results).
    """
    from concourse.bass_utils import run_bass_kernel_spmd

    nc = _get_nc()
    aw16 = aw.astype(np.float16)
    in_maps = [
        {"x": np.ascontiguousarray(aw16[c]).reshape(P, W)} for c in range(H)
    ]
    res = run_bass_kernel_spmd(nc, in_maps, list(range(H)), **run_kwargs)
    # bm[p, t*NBLK+b] -> row-major [TGT, NBLK]: row = 16p + t
    bms = [res.results[c]["bm"].reshape(TGT, NBLK) for c in range(H)]
    return np.stack(bms), res


def _exact_argmax(aw, bm):
    """Exact first-occurrence np.argmax(aw, -1) from fp16 block maxima.

    fp16 rounding is monotone, so every element equal to the f32 row max
    lives in a block whose fp16 block max ties the row's fp16 max. Scanning
    the tied blocks in ascending order preserves first-occurrence order.
    """
    rowmax = bm.max(-1, keepdims=True)
    mask = bm == rowmax  # [H, TGT, NBLK] candidate blocks
    cmax = int(mask.sum(-1).max())
    # candidate block indices in ascending order, padded with non-candidates
    order = np.argsort(~mask, axis=-1, kind="stable")[..., :cmax]
    valid = np.take_along_axis(mask, order, -1)
    blocks = aw.reshape(H, TGT, NBLK, BLK)
    win = np.take_along_axis(blocks, order[..., None], axis=2)  # [H,T,cmax,BLK]
    win = np.where(valid[..., None], win, -np.inf).reshape(H, TGT, cmax * BLK)
    j = win.argmax(-1)
    b = np.take_along_axis(order, (j // BLK)[..., None], -1)[..., 0]
    return b * BLK + j % BLK


def kernel(attention_weight):
    aw = np.asarray(attention_weight)
    assert aw.shape == (H, TGT, SRC), aw.shape
    aw = aw.astype(np.float32, copy=False)

    try:
        bm, _ = run_device(aw)
    except Exception as e:  # device path failed: fall back to host blockmax
        import traceback

        traceback.print_exc()
        print(f"WARNING: device path failed ({e!r}); falling back to numpy")
        bm = aw.astype(np.float16).reshape(H, TGT, NBLK, BLK).max(-1)

    cand = _exact_argmax(aw, bm)  # [H, TGT]
    present = np.zeros((H, SRC), np.float32)
    present[np.arange(H)[:, None], cand] = 1.0
    counting = present.sum(axis=0)

    if counting.max() <= K_THRESH:
        return np.broadcast_to(aw[SAMPLED_T0], aw.shape).copy()
    return aw


# revision 16
# speedup vs baseline: 1.0496x; 1.0231x over previous
"""Trainium2 Bass kernel for nn_SamplePolicy_14886356648064.

Reference semantics (T=4 resample rounds, K=4 vote threshold, H=8 heads):
  each round: per-head argmax over src -> presence vector per head ->
  counting = sum of presence over heads -> trigger = counting.max() <= K ->
  if trigger, replace all heads with head `sampled_t` (broadcast).

Exact algebraic collapse of the T-loop: only round 0's trigger and sampled_0
matter, for ANY input -> output is either the input (no trigger) or
broadcast(aw[3]).  sampled_0 = jax.random.randint(fold_in(key(42), 0), (),
0, 7) == 3 (threefry, platform independent).

Device work: per-row 128-wide block maxima of the fp16-cast input, one head
per core. fp16 rounding is monotone (x <= y => f16(x) <= f16(y)), so the f32
row argmax always lives in a block whose fp16 block-max ties the row's fp16
max. The device streams the 16MB fp16 head slice once; the host then scans
only the tied candidate blocks (~2 of 32 per row for uniform data) in f32
for the exact first-occurrence argmax, and runs the tiny vote logic.

Measured HW model driving the design (all 128 partitions in parallel):
  DMA: two HWDGE rings, ~425 GB/s aggregate when both active (sync ring
    ~236, scalar ring ~189); 16 DMA engines at ~26.8 GB/s each, 100% busy.
  DVE tensor_reduce: 1.04 ns/input-elem + 150ns, ANY dtype (no fast mode).
  DVE tensor_tensor max: 0.52 ns/output-elem + 150ns for packed 2-byte
    dtypes (2x_1p mode); pool/scalar engines cannot run tensor ops on TRN2.
Block maxima = fold tree (pairwise tensor_max halving block width 4x) + one
small reduce: 5.3us per 2MB chunk vs 9.1us for direct reduces.

Layout: head slice [2048, 4096] fp16 viewed as x[p, k] = [128, 65536]
(partition p holds rows 16p..16p+15 contiguously; k = t*4096 + c). Chunks
are arbitrary [c0, c0+clen) column ranges - no tile boundaries - and the
block-max output bm[p, t*32+b] reshapes straight to row-major [2048, 32].
The full 16MB fits in SBUF (128KB/partition), so every DMA job owns a
dedicated region: no write-after-read hazards anywhere.

Schedule: ring sizes/order chosen by an offline search against the
calibrated two-ring rate model so chunk completions (ring-FIFO) match the
vector engine's consumption order with minimal starvation under +-10% ring
rate skew. Small chunks first (vector starts ~10.5us), one big final chunk
(vector is backlogged at the end anyway; big chunks cost least per element).
"""

import numpy as np

H = 8
TGT = 2048
SRC = 4096
P = 128            # SBUF partitions
RPP = 16           # rows per partition
W = RPP * SRC      # 65536 elements per partition
NBLK = 32          # block maxima per original row
BLK = SRC // NBLK  # 128
NBLK_ALL = W // BLK  # 512 block maxima per partition
K_THRESH = 4
SAMPLED_T0 = 3

# (ring, clen) in VECTOR (predicted-arrival) order; c0 = running sum.
# The two HWDGE queues split DMA-engine slots round-robin per DESCRIPTOR, so
# bytes split proportional to descriptor size; a lone queue streams ~330-400
# GB/s, both together ~425. Ring A opens with four 1MB chunks streamed solo
# (ring B's first issue is semaphore-gated on A1's completion) so the vector
# engine starts folding ~12.5us; the rings then interleave 2MB chunks with a
# half-chunk offset (no lumpy pair completions).
SCHEDULE = [
    ("A", 4096), ("A", 4096), ("A", 4096), ("A", 4096),
    ("B", 4096), ("A", 8192), ("B", 8192), ("A", 8192),
    ("B", 8192), ("B", 8192), ("B", 4096),
]
GATE_B_ON = 1      # ring B's first dma_start waits for this job's load sem
# fold depth per chunk size (then one reduce over the remaining width)
FOLDK = {16384: 4, 8192: 4, 4096: 3, 2048: 2, 1024: 1}
# gpsimd tick train: engines re-evaluate blocked semaphore waits only when
# some new semaphore update arrives; a steady stream of dummy sem_incs
# bounds that wake-up latency at one tick period (~0.5us).
TICKS = 95
TICK_SETS = 1

JOBS = []          # (ring, c0, clen) in vector order
_g = 0
for _ring, _clen in SCHEDULE:
    JOBS.append((_ring, _g, _clen))
    _g += _clen
assert _g == W
NJOBS = len(JOBS)
FINAL_C0 = JOBS[-1][1]          # bulk store covers blocks [0, FINAL_C0/BLK)

_cache = {}


def _build_nc():
    """Raw Bass program, one head per core."""
    from contextlib import ExitStack

    import concourse.bass as bass
    import concourse.mybir as mybir

    nc = bass.Bass()
    f16 = mybir.dt.float16
    x = nc.declare_dram_parameter("x", [P, W], f16, isOutput=False)
    # bm[p, t*NBLK + b] = max of block b of row 16p + t
    bm = nc.declare_dram_parameter("bm", [P, NBLK_ALL], f16, isOutput=True)

    with ExitStack() as ctx:
        tiles = ctx.enter_context(nc.sbuf_tensor([P, W], f16))
        scra = ctx.enter_context(nc.sbuf_tensor([P, 8192], f16))
        scrb = ctx.enter_context(nc.sbuf_tensor([P, 4096], f16))
        bmsb = ctx.enter_context(nc.sbuf_tensor([P, NBLK_ALL], f16))
        tick = ctx.enter_context(nc.sbuf_tensor([1, 512], f16))
        # one load semaphore per RING, cumulative thresholds (16 per job):
        # a threshold of 16*k implies the ring's first k jobs all completed,
        # regardless of increment interleaving across jobs.
        s_ld = {r: ctx.enter_context(nc.semaphore(f"s_ld{r}")) for r in "AB"}
        s_red = ctx.enter_context(nc.semaphore("s_red"))
        s_out = ctx.enter_context(nc.semaphore("s_out"))
        s_tick = ctx.enter_context(nc.semaphore("s_tick"))
        block = ctx.enter_context(nc.Block())

        # per-job (ring, #jobs on that ring up to and including this one)
        RANK = []
        cnt = {"A": 0, "B": 0}
        for r, _, _ in JOBS:
            cnt[r] += 1
            RANK.append((r, cnt[r]))
        gate_ring, gate_rank = RANK[GATE_B_ON]

        def issue_loads(eng, ring):
            first = True
            for j, (r, c0, clen) in enumerate(JOBS):
                if r != ring:
                    continue
                if ring == "B" and first:
                    eng.wait_ge(s_ld[gate_ring], 16 * gate_rank)
                    first = False
                eng.dma_start(
                    out=tiles[:, c0 : c0 + clen],
                    in_=x[:, c0 : c0 + clen],
                ).then_inc(s_ld[r], 16)

        @block.sync
        def _(sync):
            issue_loads(sync, "A")
            sync.wait_ge(s_out, 32)

        @block.scalar
        def _(scalar):
            issue_loads(scalar, "B")
            # bulk store overlaps the final chunk's fold train
            scalar.wait_ge(s_red, NJOBS - 1)
            scalar.dma_start(
                out=bm[:, : FINAL_C0 // BLK], in_=bmsb[:, : FINAL_C0 // BLK]
            ).then_inc(s_out, 16)
            scalar.wait_ge(s_red, NJOBS)
            scalar.dma_start(
                out=bm[:, FINAL_C0 // BLK :], in_=bmsb[:, FINAL_C0 // BLK :]
            ).then_inc(s_out, 16)

        @block.gpsimd
        def _(gpsimd):
            # pure ticker: never waits, just emits a sem update every ~0.5us
            for _ in range(TICKS):
                for _ in range(TICK_SETS):
                    nc.gpsimd.memset(tick[:, :], 0.0)
                gpsimd.sem_inc(s_tick, 1)

        @block.vector
        def _(vector):
            for j, (r, c0, clen) in enumerate(JOBS):
                vector.wait_ge(s_ld[r], 16 * RANK[j][1])
                nblk = clen // BLK
                # fold tree: halve block width FOLDK times, ping-pong scratch
                src = tiles[:, c0 : c0 + clen]
                w = BLK
                for step in range(FOLDK[clen]):
                    dst = (scra if step % 2 == 0 else scrb)[:, : nblk * w // 2]
                    sv = src.rearrange("p (b c) -> p b c", c=w)
                    nc.vector.tensor_max(
                        out=dst.rearrange("p (b c) -> p b c", c=w // 2),
                        in0=sv[:, :, : w // 2],
                        in1=sv[:, :, w // 2 :],
                    )
                    src, w = dst, w // 2
                # finish with one reduce over the remaining width
                nc.vector.reduce_max(
                    out=bmsb[:, c0 // BLK : (c0 + clen) // BLK],
                    in_=src.rearrange("p (b c) -> p b c", c=w),
                    axis=mybir.AxisListType.X,
                ).then_inc(s_red, 1)

    return nc


def _get_nc():
    if "nc" not in _cache:
        _cache["nc"] = _build_nc()
    return _cache["nc"]


def run_device(aw, **run_kwargs):
    """Run the per-head fp16 block-max kernel on 8 cores.

    Takes the full f32 [H, TGT, SRC] tensor; returns ([H, TGT, NBLK] fp16
    block maxima of the fp16-cast data, results).
    """
    from concourse.bass_utils import run_bass_kernel_spmd

    nc = _get_nc()
    aw16 = aw.astype(np.float16)
    in_maps = [
        {"x": np.ascontiguousarray(aw16[c]).reshape(P, W)} for c in range(H)
    ]
    res = run_bass_kernel_spmd(nc, in_maps, list(range(H)), **run_kwargs)
    # bm[p, t*NBLK+b] -> row-major [TGT, NBLK]: row = 16p + t
    bms = [res.results[c]["bm"].reshape(TGT, NBLK) for c in range(H)]
    return np.stack(bms), res


def _exact_argmax(aw, bm):
    """Exact first-occurrence np.argmax(aw, -1) from fp16 block maxima.

    fp16 rounding is monotone, so every element equal to the f32 row max
    lives in a block whose fp16 block max ties the row's fp16 max. Scanning
    the tied blocks in ascending order preserves first-occurrence order.
    """
    rowmax = bm.max(-1, keepdims=True)
    mask = bm == rowmax  # [H, TGT, NBLK] candidate blocks
    cmax = int(mask.sum(-1).max())
    # candidate block indices in ascending order, padded with non-candidates
    order = np.argsort(~mask, axis=-1, kind="stable")[..., :cmax]
    valid = np.take_along_axis(mask, order, -1)
    blocks = aw.reshape(H, TGT, NBLK, BLK)
    win = np.take_along_axis(blocks, order[..., None], axis=2)  # [H,T,cmax,BLK]
    win = np.where(valid[..., None], win, -np.inf).reshape(H, TGT, cmax * BLK)
    j = win.argmax(-1)
    b = np.take_along_axis(order, (j // BLK)[..., None], -1)[..., 0]
    return b * BLK + j % BLK


def kernel(attention_weight):
    aw = np.asarray(attention_weight)
    assert aw.shape == (H, TGT, SRC), aw.shape
    aw = aw.astype(np.float32, copy=False)

    try:
        bm, _ = run_device(aw)
    except Exception as e:  # device path failed: fall back to host blockmax
        import traceback

        traceback.print_exc()
        print(f"WARNING: device path failed ({e!r}); falling back to numpy")
        bm = aw.astype(np.float16).reshape(H, TGT, NBLK, BLK).max(-1)

    cand = _exact_argmax(aw, bm)  # [H, TGT]
    present = np.zeros((H, SRC), np.float32)
    present[np.arange(H)[:, None], cand] = 1.0
    counting = present.sum(axis=0)

    if counting.max() <= K_THRESH:
        return np.broadcast_to(aw[SAMPLED_T0], aw.shape).copy()
    return aw


# revision 17
# speedup vs baseline: 1.1097x; 1.0572x over previous
"""Trainium2 Bass kernel for nn_SamplePolicy_14886356648064.

Reference semantics (T=4 resample rounds, K=4 vote threshold, H=8 heads):
  each round: per-head argmax over src -> presence vector per head ->
  counting = sum of presence over heads -> trigger = counting.max() <= K ->
  if trigger, replace all heads with head `sampled_t` (broadcast).

Exact algebraic collapse of the T-loop: only round 0's trigger and sampled_0
matter, for ANY input -> output is either the input (no trigger) or
broadcast(aw[3]).  sampled_0 = jax.random.randint(fold_in(key(42), 0), (),
0, 7) == 3 (threefry, platform independent).

Device work: per-row 128-wide block maxima of the fp16-cast input, one head
per core. fp16 rounding is monotone (x <= y => f16(x) <= f16(y)), so the f32
row argmax always lives in a block whose fp16 block-max ties the row's fp16
max. The device streams the 16MB fp16 head slice once; the host then scans
only the tied candidate blocks (~2 of 32 per row for uniform data) in f32
for the exact first-occurrence argmax, and runs the tiny vote logic.

Measured HW model driving the design (all 128 partitions in parallel):
  DMA: two HWDGE rings, ~425 GB/s aggregate when both active (sync ring
    ~236, scalar ring ~189); 16 DMA engines at ~26.8 GB/s each, 100% busy.
  DVE tensor_reduce: 1.04 ns/input-elem + 150ns, ANY dtype (no fast mode).
  DVE tensor_tensor max: 0.52 ns/output-elem + 150ns for packed 2-byte
    dtypes (2x_1p mode); pool/scalar engines cannot run tensor ops on TRN2.
Block maxima = fold tree (pairwise tensor_max halving block width 4x) + one
small reduce: 5.3us per 2MB chunk vs 9.1us for direct reduces.

Layout: head slice [2048, 4096] fp16 viewed as x[p, k] = [128, 65536]
(partition p holds rows 16p..16p+15 contiguously; k = t*4096 + c). Chunks
are arbitrary [c0, c0+clen) column ranges - no tile boundaries - and the
block-max output bm[p, t*32+b] reshapes straight to row-major [2048, 32].
The full 16MB fits in SBUF (128KB/partition), so every DMA job owns a
dedicated region: no write-after-read hazards anywhere.

Schedule: ring sizes/order chosen by an offline search against the
calibrated two-ring rate model so chunk completions (ring-FIFO) match the
vector engine's consumption order with minimal starvation under +-10% ring
rate skew. Small chunks first (vector starts ~10.5us), one big final chunk
(vector is backlogged at the end anyway; big chunks cost least per element).
"""

import numpy as np

H = 8
TGT = 2048
SRC = 4096
P = 128            # SBUF partitions
RPP = 16           # rows per partition
W = RPP * SRC      # 65536 elements per partition
NBLK = 32          # block maxima per original row
BLK = SRC // NBLK  # 128
NBLK_ALL = W // BLK  # 512 block maxima per partition
K_THRESH = 4
SAMPLED_T0 = 3

# (ring, clen) in VECTOR (predicted-arrival) order; c0 = running sum.
# The two HWDGE queues split DMA-engine slots round-robin per DESCRIPTOR, so
# bytes split proportional to descriptor size; a lone queue streams ~330-400
# GB/s, both together ~425. Ring A opens with four 1MB chunks streamed
# mostly solo (ring B's first issue is semaphore-gated on A1's completion,
# and B's descriptors only start ~3us after the issue) so the vector engine
# starts folding ~12.5us; ring B opens and closes with a 1MB chunk so its
# first arrival lands before the vector drains the ramp; the rings then
# interleave 2MB chunks with a half-chunk offset (no lumpy pair
# completions).
SCHEDULE = [
    ("A", 4096), ("A", 4096), ("A", 4096), ("A", 4096),
    ("B", 4096), ("A", 8192), ("B", 8192), ("A", 8192),
    ("B", 8192), ("B", 8192), ("B", 4096),
]
GATE_B_ON = 1      # ring B's first dma_start waits for this job's load sem
# fold depth per chunk size (then one reduce over the remaining width)
FOLDK = {16384: 4, 8192: 4, 4096: 3, 2048: 2, 1024: 1}
# gpsimd tick train: engines re-evaluate blocked semaphore waits only when
# some new semaphore update arrives; a steady stream of dummy sem_incs
# bounds that wake-up latency at one tick period (~0.5us).
TICKS = 95
TICK_SETS = 1

JOBS = []          # (ring, c0, clen) in vector order
_g = 0
for _ring, _clen in SCHEDULE:
    JOBS.append((_ring, _g, _clen))
    _g += _clen
assert _g == W
NJOBS = len(JOBS)
FINAL_C0 = JOBS[-1][1]          # bulk store covers blocks [0, FINAL_C0/BLK)

_cache = {}


def _build_nc():
    """Raw Bass program, one head per core."""
    from contextlib import ExitStack

    import concourse.bass as bass
    import concourse.mybir as mybir

    nc = bass.Bass()
    f16 = mybir.dt.float16
    x = nc.declare_dram_parameter("x", [P, W], f16, isOutput=False)
    # bm[p, t*NBLK + b] = max of block b of row 16p + t
    bm = nc.declare_dram_parameter("bm", [P, NBLK_ALL], f16, isOutput=True)

    with ExitStack() as ctx:
        tiles = ctx.enter_context(nc.sbuf_tensor([P, W], f16))
        scra = ctx.enter_context(nc.sbuf_tensor([P, 8192], f16))
        scrb = ctx.enter_context(nc.sbuf_tensor([P, 4096], f16))
        bmsb = ctx.enter_context(nc.sbuf_tensor([P, NBLK_ALL], f16))
        tick = ctx.enter_context(nc.sbuf_tensor([1, 512], f16))
        # one load semaphore per RING, cumulative thresholds (16 per job):
        # a threshold of 16*k implies the ring's first k jobs all completed,
        # regardless of increment interleaving across jobs.
        s_ld = {r: ctx.enter_context(nc.semaphore(f"s_ld{r}")) for r in "AB"}
        s_red = ctx.enter_context(nc.semaphore("s_red"))
        s_out = ctx.enter_context(nc.semaphore("s_out"))
        s_tick = ctx.enter_context(nc.semaphore("s_tick"))
        block = ctx.enter_context(nc.Block())

        # per-job (ring, #jobs on that ring up to and including this one)
        RANK = []
        cnt = {"A": 0, "B": 0}
        for r, _, _ in JOBS:
            cnt[r] += 1
            RANK.append((r, cnt[r]))
        gate_ring, gate_rank = RANK[GATE_B_ON]

        def issue_loads(eng, ring):
            first = True
            for j, (r, c0, clen) in enumerate(JOBS):
                if r != ring:
                    continue
                if ring == "B" and first:
                    eng.wait_ge(s_ld[gate_ring], 16 * gate_rank)
                    first = False
                eng.dma_start(
                    out=tiles[:, c0 : c0 + clen],
                    in_=x[:, c0 : c0 + clen],
                ).then_inc(s_ld[r], 16)

        @block.sync
        def _(sync):
            issue_loads(sync, "A")
            sync.wait_ge(s_out, 32)

        @block.scalar
        def _(scalar):
            issue_loads(scalar, "B")
            # bulk store overlaps the final chunk's fold train
            scalar.wait_ge(s_red, NJOBS - 1)
            scalar.dma_start(
                out=bm[:, : FINAL_C0 // BLK], in_=bmsb[:, : FINAL_C0 // BLK]
            ).then_inc(s_out, 16)
            scalar.wait_ge(s_red, NJOBS)
            scalar.dma_start(
                out=bm[:, FINAL_C0 // BLK :], in_=bmsb[:, FINAL_C0 // BLK :]
            ).then_inc(s_out, 16)

        @block.gpsimd
        def _(gpsimd):
            # pure ticker: never waits, just emits a sem update every ~0.5us
            for _ in range(TICKS):
                for _ in range(TICK_SETS):
                    nc.gpsimd.memset(tick[:, :], 0.0)
                gpsimd.sem_inc(s_tick, 1)

        @block.vector
        def _(vector):
            for j, (r, c0, clen) in enumerate(JOBS):
                vector.wait_ge(s_ld[r], 16 * RANK[j][1])
                nblk = clen // BLK
                # fold tree: halve block width FOLDK times, ping-pong scratch
                src = tiles[:, c0 : c0 + clen]
                w = BLK
                for step in range(FOLDK[clen]):
                    dst = (scra if step % 2 == 0 else scrb)[:, : nblk * w // 2]
                    sv = src.rearrange("p (b c) -> p b c", c=w)
                    nc.vector.tensor_max(
                        out=dst.rearrange("p (b c) -> p b c", c=w // 2),
                        in0=sv[:, :, : w // 2],
                        in1=sv[:, :, w // 2 :],
                    )
                    src, w = dst, w // 2
                # finish with one reduce over the remaining width
                nc.vector.reduce_max(
                    out=bmsb[:, c0 // BLK : (c0 + clen) // BLK],
                    in_=src.rearrange("p (b c) -> p b c", c=w),
                    axis=mybir.AxisListType.X,
                ).then_inc(s_red, 1)

    return nc


def _get_nc():
    if "nc" not in _cache:
        _cache["nc"] = _build_nc()
    return _cache["nc"]


def run_device(aw, **run_kwargs):
    """Run the per-head fp16 block-max kernel on 8 cores.

    Takes the full f32 [H, TGT, SRC] tensor; returns ([H, TGT, NBLK] fp16
    block maxima of the fp16-cast data, results).
    """
    from concourse.bass_utils import run_bass_kernel_spmd

    nc = _get_nc()
    aw16 = aw.astype(np.float16)
    in_maps = [
        {"x": np.ascontiguousarray(aw16[c]).reshape(P, W)} for c in range(H)
    ]
    res = run_bass_kernel_spmd(nc, in_maps, list(range(H)), **run_kwargs)
    # bm[p, t*NBLK+b] -> row-major [TGT, NBLK]: row = 16p + t
    bms = [res.results[c]["bm"].reshape(TGT, NBLK) for c in range(H)]
    return np.stack(bms), res


def _exact_argmax(aw, bm):
    """Exact first-occurrence np.argmax(aw, -1) from fp16 block maxima.

    fp16 rounding is monotone, so every element equal to the f32 row max
    lives in a block whose fp16 block max ties the row's fp16 max. Scanning
    the tied blocks in ascending order preserves first-occurrence order.
    """
    rowmax = bm.max(-1, keepdims=True)
    mask = bm == rowmax  # [H, TGT, NBLK] candidate blocks
    cmax = int(mask.sum(-1).max())
    # candidate block indices in ascending order, padded with non-candidates
    order = np.argsort(~mask, axis=-1, kind="stable")[..., :cmax]
    valid = np.take_along_axis(mask, order, -1)
    blocks = aw.reshape(H, TGT, NBLK, BLK)
    win = np.take_along_axis(blocks, order[..., None], axis=2)  # [H,T,cmax,BLK]
    win = np.where(valid[..., None], win, -np.inf).reshape(H, TGT, cmax * BLK)
    j = win.argmax(-1)
    b = np.take_along_axis(order, (j // BLK)[..., None], -1)[..., 0]
    return b * BLK + j % BLK


def kernel(attention_weight):
    aw = np.asarray(attention_weight)
    assert aw.shape == (H, TGT, SRC), aw.shape
    aw = aw.astype(np.float32, copy=False)

    try:
        bm, _ = run_device(aw)
    except Exception as e:  # device path failed: fall back to host blockmax
        import traceback

        traceback.print_exc()
        print(f"WARNING: device path failed ({e!r}); falling back to numpy")
        bm = aw.astype(np.float16).reshape(H, TGT, NBLK, BLK).max(-1)

    cand = _exact_argmax(aw, bm)  # [H, TGT]
    present = np.zeros((H, SRC), np.float32)
    present[np.arange(H)[:, None], cand] = 1.0
    counting = present.sum(axis=0)

    if counting.max() <= K_THRESH:
        return np.broadcast_to(aw[SAMPLED_T0], aw.shape).copy()
    return aw
